# revision 1
# baseline (speedup 1.0000x reference)
"""Trainium2 Bass kernel for nn_CostAggLaplace (cost-volume aggregation with
Laplacian divergence weighting and a tiny 1x1x1-conv MLP).

Strategy:
  - Shard the H (image rows) axis across the 8 NeuronCores: core q computes
    output rows [24q, 24q+24) for all (view, batch, depth) units. Every core
    runs the SAME program (shift constants are identical across cores);
    cores differ only in which row-slice of the inputs they receive.
  - Host detects the warp structure: proj = src_proj @ inv(ref_proj) with
    rot = lam*I and trans_z ~ 0 makes the homography a pure per-depth 2D
    translation, so bilinear warping becomes a 4-tap stencil with constant
    per-depth weights. The host pre-tiles each source view into per-partition
    halo'd windows sized by that view's shift envelope (all six images stay
    resident in SBUF); the integer shift becomes a free-dim AP offset and the
    fractional weights become 4 accumulating diagonal fp32r (TF32) matmuls on
    the tensor engine, plus a 5th matmul subtracting (ref + 1e-8) so the
    divergence is exp(-3*|psum * refrecip|).
  - The 1x1x1 conv MLP (8->16->8->1, BN folded) runs as block-diagonal
    matmuls with 16 pixel-blocks packed along the contraction dim. Sigmoid is
    computed as 0.5*tanh(x/2)+0.5 and the final division via batched Ln/Exp
    so the scalar engine never leaves its loaded activation-table sets.
  - Elementwise work is spread across DVE/ACT/GPSIMD for balance; the
    per-depth diagonal weight matrices are precomputed on the host and DMA'd.
  - If the warp structure does not hold (arbitrary projection matrices),
    fall back to an exact numpy port of the reference.

Partition layout for all spatial tiles: p = (yt*2 + xh)*8 + c with yt in
[0,8) (3-row tiles), xh in [0,2) (128-col halves), c channel. Free dim is
[yl(3), x(128)] = 384 elements.
"""

import math
import numpy as np

V, B, C, H, W, D = 4, 2, 8, 192, 256, 32
NCORES = 8
RPC = H // NCORES            # rows per core = 24
YT, XH = 8, 2                # yt tiles x column halves
RY = RPC // YT               # rows per yt tile = 3
XW = W // XH                 # cols per half = 128
NFREE = RY * XW              # 384
NVIEW = V - 1                # 3 source views
BN_EPS = 1e-5
LAMBDA = 3.0

_COMPILE_CACHE = {}
# populated after each device run: {"exec_time_ns": ..., "profile_json": ...}
LAST_RUN_INFO = {}


# ----------------------------------------------------------------------------
# host-side math helpers
# ----------------------------------------------------------------------------

def _fold_bn(w, g, bta, m, v):
    """BN(conv(x)) == diag(s) @ w @ x + (bta - m*s), s = g/sqrt(v+eps)."""
    s = (g / np.sqrt(v + BN_EPS)).astype(np.float64)
    wf = (s[:, None] * w.astype(np.float64))
    cf = (bta.astype(np.float64) - m.astype(np.float64) * s)
    return wf.astype(np.float32), cf.astype(np.float32)


def _analyze(ref_proj, src_projs, depth_hypos):
    """Return (ok, shifts) where shifts[i][b][d] = (sx, sy) pixel shift,
    or (False, None) when the warp is not a pure 2D translation."""
    try:
        inv_ref = np.linalg.inv(ref_proj.astype(np.float64))  # [B,4,4]
    except np.linalg.LinAlgError:
        return False, None
    P = src_projs.astype(np.float64) @ inv_ref[None]          # [NV,B,4,4]
    if not np.all(np.isfinite(P)) or not np.all(np.isfinite(depth_hypos)):
        return False, None
    rot = P[:, :, :3, :3]
    trans = P[:, :, :3, 3]
    lam = rot[:, :, 0, 0]
    scale = np.maximum(np.abs(P).max(), 1e-12)
    dev = np.abs(rot - lam[:, :, None, None] * np.eye(3)).max()
    if dev > 1e-6 * scale or np.abs(trans[:, :, 2]).max() > 1e-9 * scale:
        return False, None
    if np.abs(lam).min() < 1e-12 * scale:
        return False, None
    shifts = np.empty((NVIEW, B, D, 2), np.float64)
    dep = depth_hypos.astype(np.float64)                      # [B,D]
    for i in range(NVIEW):
        for b in range(B):
            pz = lam[i, b] * dep[b]                           # [D]
            with np.errstate(divide="ignore", invalid="ignore"):
                sx = trans[i, b, 0] / pz
                sy = trans[i, b, 1] / pz
            shifts[i, b, :, 0] = sx
            shifts[i, b, :, 1] = sy
    if not np.all(np.isfinite(shifts)):
        # infinite shift => fully out of image => handled as zero weights,
        # encode with a huge sentinel
        shifts = np.nan_to_num(shifts, nan=1e9, posinf=1e9, neginf=-1e9)
    return True, shifts


def _np_reference(features, depth_hypos, ref_proj, src_projs, w1, g1, b1, m1,
                  v1, w2, g2, b2, m2, v2, w3, b3):
    """Exact numpy port of the jax reference (fallback path)."""
    f32 = np.float32

    def bn(x, g, bta, m, v):
        sh = (1, -1, 1, 1, 1)
        return (x - m.reshape(sh)) * (g / np.sqrt(v + BN_EPS)).reshape(sh) \
            + bta.reshape(sh)

    def bilinear_sample(fea, px, py):
        Bb, Cc, Hh, Ww = fea.shape
        x0 = np.floor(px); y0 = np.floor(py)
        dx = px - x0; dy = py - y0
        fea_flat = fea.reshape(Bb, Cc, Hh * Ww)

        def gather(xf, yf):
            valid = (xf >= 0) & (xf <= Ww - 1) & (yf >= 0) & (yf <= Hh - 1)
            xi = np.clip(xf, 0, Ww - 1).astype(np.int32)
            yi = np.clip(yf, 0, Hh - 1).astype(np.int32)
            idx = (yi * Ww + xi).reshape(Bb, -1)
            g = np.stack([np.take(fea_flat[bb], idx[bb], axis=1)
                          for bb in range(Bb)])
            return (g.reshape(Bb, Cc, px.shape[1], px.shape[2])
                    * valid[:, None].astype(fea.dtype))
        v00 = gather(x0, y0); v01 = gather(x0 + 1, y0)
        v10 = gather(x0, y0 + 1); v11 = gather(x0 + 1, y0 + 1)
        dx = dx[:, None]; dy = dy[:, None]
        return (v00 * (1 - dx) * (1 - dy) + v01 * dx * (1 - dy)
                + v10 * (1 - dx) * dy + v11 * dx * dy)

    def homo_warp(src_fea, src_proj, ref_proj_, depth):
        Bb, Cc, Hh, Ww = src_fea.shape
        Dd = depth.shape[1]
        proj = src_proj @ np.linalg.inv(ref_proj_)
        rot, trans = proj[:, :3, :3], proj[:, :3, 3]
        yy, xx = np.meshgrid(np.arange(Hh, dtype=src_fea.dtype),
                             np.arange(Ww, dtype=src_fea.dtype), indexing="ij")
        xyz = np.stack([xx.ravel(), yy.ravel(),
                        np.ones(Hh * Ww, src_fea.dtype)], 0)
        rot_xyz = np.einsum("bij,jk->bik", rot, xyz)
        pxyz = (rot_xyz[:, :, None, :] * depth[:, None, :, None]
                + trans[:, :, None, None])
        px = pxyz[:, 0] / pxyz[:, 2]
        py = pxyz[:, 1] / pxyz[:, 2]
        return bilinear_sample(src_fea, px, py).reshape(Bb, Cc, Dd, Hh, Ww)

    def sigmoid(x):
        return 1.0 / (1.0 + np.exp(-x))

    def weight_probs(vol):
        h = np.maximum(bn(np.einsum("oc,bcdhw->bodhw", w1, vol),
                          g1, b1, m1, v1), 0)
        h = np.maximum(bn(np.einsum("oc,bcdhw->bodhw", w2, h),
                          g2, b2, m2, v2), 0)
        h = np.einsum("oc,bcdhw->bodhw", w3, h) + b3.reshape(1, -1, 1, 1, 1)
        return sigmoid(h)

    ref_volume = features[0][:, :, None]
    cost_volume, weight_sum = f32(0.0), f32(0.0)
    for i in range(src_projs.shape[0]):
        src_vol = homo_warp(features[i + 1], src_projs[i], ref_proj,
                            depth_hypos)
        vol = np.exp(-LAMBDA * np.abs(1.0 - src_vol / (ref_volume + 1e-8)))
        w = weight_probs(vol)
        cost_volume = cost_volume + vol * w
        weight_sum = weight_sum + w
    return (cost_volume / weight_sum).astype(np.float32)


# ----------------------------------------------------------------------------
# unit constants (shared by program builder and host input prep)
# ----------------------------------------------------------------------------

def _unit_constants(shifts):
    """Per (d, b, i): integer shifts (ax, ay) and 4 corner weights.
    Weight order: (dy,dx) = (0,0),(0,1),(1,0),(1,1)."""
    units = []
    for d in range(D):
        for b in range(B):
            for i in range(NVIEW):
                sx, sy = shifts[i, b, d, 0], shifts[i, b, d, 1]
                ax = math.floor(sx) if abs(sx) < 1e6 else 0
                ay = math.floor(sy) if abs(sy) < 1e6 else 0
                fx = sx - ax
                fy = sy - ay
                w00 = (1 - fy) * (1 - fx)
                w01 = (1 - fy) * fx
                w10 = fy * (1 - fx)
                w11 = fy * fx
                # clamp: if the sampled window cannot intersect the image,
                # zero the weights and clamp the read window
                if sx > W or sx < -(W + 1) or sy > H or sy < -(H + 1) \
                        or abs(sx) >= 1e6 or abs(sy) >= 1e6:
                    ax, ay = 0, 0
                    w00 = w01 = w10 = w11 = 0.0
                units.append((i, b, d, int(ax), int(ay),
                              float(w00), float(w01), float(w10), float(w11)))
    return units


def _envelopes(units):
    """Per (i, b): (ax_lo, ax_hi, ay_lo, ay_hi) over that view's units."""
    env = {}
    for (i, b, _d, ax, ay, *_w) in units:
        lo = env.get((i, b))
        if lo is None:
            env[(i, b)] = [ax, ax, ay, ay]
        else:
            lo[0] = min(lo[0], ax); lo[1] = max(lo[1], ax)
            lo[2] = min(lo[2], ay); lo[3] = max(lo[3], ay)
    return env


def _geom(env_ib):
    ax_lo, ax_hi, ay_lo, ay_hi = env_ib
    yle = RY + 1 + (ay_hi - ay_lo)
    xe = XW + 1 + (ax_hi - ax_lo)
    return yle, xe


def _tileize(arr_bchw):
    """[B?, C, RPC, W] -> [B?, 128, NFREE] in partition layout
    p=(yt*2+xh)*8+c, free [yl, x]."""
    a = arr_bchw.reshape(-1, C, YT, RY, XH, XW)
    a = a.transpose(0, 2, 4, 1, 3, 5)          # (n, yt, xh, c, yl, x)
    return np.ascontiguousarray(a.reshape(-1, 128, NFREE))


def _untileize(arr_p, n):
    """[n, 128, NFREE] -> [n, C, RPC, W]"""
    a = arr_p.reshape(n, YT, XH, C, RY, XW)
    a = a.transpose(0, 3, 1, 4, 2, 5)
    return a.reshape(n, C, RPC, W)


# ----------------------------------------------------------------------------
# device program
# ----------------------------------------------------------------------------

def _dg_plan(units):
    """Distinct corner-weight sets in program order (dedup across b within
    each d, mirroring the builder's cache)."""
    umap = {(u[0], u[1], u[2]): u for u in units}
    plan = []
    unit_idx = {}
    for d in range(D):
        cache = {}
        for b in range(B):
            for i in range(NVIEW):
                (_i, _b, _d, _ax, _ay, w00, w01, w10, w11) = umap[(i, b, d)]
                ck = (i, round(w00, 9), round(w01, 9), round(w10, 9),
                      round(w11, 9))
                if ck not in cache:
                    cache[ck] = len(plan)
                    plan.append((w00, w01, w10, w11))
                unit_idx[(i, b, d)] = cache[ck]
    return plan, unit_idx


def _build_program(units, env, hoist=True):
    import concourse.bass as bass
    import concourse.mybir as mybir
    from concourse.tile import TileContext
    from concourse.vector_clock import ScopedClock

    class SplitDrainTC(TileContext):
        """This walrus build only accepts 1 sync wait per CTRL instruction;
        spread the end-of-kernel drain waits over multiple nops."""

        def _drain_and_barrier(self, tick_clock, wait_clock):
            nop0 = self.nc.sync.nop(nofuse=True)
            wait_clock.add_sem_waits(
                nop0.ins, ScopedClock({None: tick_clock.global_clock}))
            si = nop0.ins.sync_info
            waits = list(si.on_wait) if si is not None and si.on_wait else []
            upds = list(si.on_update) if si is not None and si.on_update else []
            if len(waits) > 1:
                nop0.ins.sync_info = mybir.SyncInfo(
                    on_wait=waits[:1], on_update=upds)
                for wv in waits[1:]:
                    nop = self.nc.sync.nop(nofuse=True)
                    nop.ins.sync_info = mybir.SyncInfo(
                        on_wait=[wv], on_update=[])
            self.nc.sync.drain()
            self.nc.all_engine_barrier()
            popped = self.nc._tile_sem_poison_stack.pop()
            assert popped is self._sem_poison
            self.nc.clear_and_free_semaphores(
                list(self.sems.allocated().values()))
            self.nc.all_engine_barrier()

    f32 = mybir.dt.float32
    f32r = mybir.dt.float32r
    A = mybir.AluOpType
    AF = mybir.ActivationFunctionType

    umap = {(u[0], u[1], u[2]): u for u in units}
    geo = {k: _geom(v) for k, v in env.items()}

    nc = bass.Bass()
    wimg_d = {}
    for (i, b), (yle, xe) in sorted(geo.items()):
        wimg_d[(i, b)] = nc.dram_tensor(f"wimg_{i}_{b}", [128, yle * xe],
                                        f32r, kind="ExternalInput")
    refr = nc.dram_tensor("refr", [B, 128, NFREE], f32, kind="ExternalInput")
    mlpw = nc.dram_tensor("mlpw", [6, 128, 128], f32r, kind="ExternalInput")
    refp = nc.dram_tensor("refp", [B, 128, NFREE], f32r,
                          kind="ExternalInput")
    biasd = nc.dram_tensor("biasd", [4, 128], f32, kind="ExternalInput")
    identd = nc.dram_tensor("identd", [128, 128], f32, kind="ExternalInput")
    dg_plan, dg_unit_idx = _dg_plan(units)
    dgd = nc.dram_tensor("dgd", [len(dg_plan), 128, 4 * 128], f32r,
                         kind="ExternalInput")
    outd = nc.dram_tensor("out", [B, D, 128, NFREE], f32,
                          kind="ExternalOutput")

    with SplitDrainTC(nc) as tc:
        with (
            tc.tile_pool(name="const", bufs=1) as cpool,
            tc.tile_pool(name="dg", bufs=8) as dgpool,
            tc.tile_pool(name="dg0", bufs=2) as dg0pool,
            tc.tile_pool(name="work", bufs=5) as wpool,
            tc.tile_pool(name="acc", bufs=18) as apool,
            tc.tile_pool(name="fin", bufs=5) as fpool,
            tc.tile_pool(name="psA", bufs=1, space="PSUM") as psa_pool,
            tc.tile_pool(name="psBC", bufs=2, space="PSUM") as psbc_pool,
            tc.tile_pool(name="psD", bufs=1, space="PSUM") as psd_pool,
            tc.tile_pool(name="psE", bufs=2, space="PSUM") as pse_pool,
        ):
            # resident constants
            refr_t = cpool.tile([128, B * NFREE], f32, tag="refr")
            for b in range(B):
                nc.sync.dma_start(out=refr_t[:, b * NFREE:(b + 1) * NFREE],
                                  in_=refr[b])
            lhs = cpool.tile([128, 6 * 128], f32r, tag="mlp")
            for j in range(6):
                nc.sync.dma_start(out=lhs[:, j * 128:(j + 1) * 128],
                                  in_=mlpw[j])
            refp_t = cpool.tile([128, B * NFREE], f32r, tag="refp")
            for b in range(B):
                nc.sync.dma_start(out=refp_t[:, b * NFREE:(b + 1) * NFREE],
                                  in_=refp[b])
            bias_t = cpool.tile([128, 4], f32, tag="bias")
            nc.sync.dma_start(
                out=bias_t[:],
                in_=bass.AP(tensor=biasd[:].tensor, offset=0,
                            ap=[[1, 128], [128, 4]]))
            ident = cpool.tile([128, 128], f32, tag="ident")
            nc.sync.dma_start(out=ident[:], in_=identd[:])
            # resident halo'd source images (one per (view, batch))
            imgs = {}
            for (i, b), (yle, xe) in sorted(geo.items()):
                img_ib = cpool.tile([128, yle * xe], f32r, tag=f"img{i}{b}")
                nc.sync.dma_start(out=img_ib[:], in_=wimg_d[(i, b)][:])
                imgs[(i, b)] = img_ib

            def lhs_r(j):
                return lhs[:, j * 128:(j + 1) * 128]

            pending = []
            for d in range(D):
                dg_cache = {}
                for b in range(B):
                    acc_c = apool.tile([128, NFREE], f32, tag="acc_c")
                    acc_w = apool.tile([128, NFREE], f32, tag="acc_w")
                    pending.append((b, d, acc_c, acc_w))
                    for i in range(NVIEW):
                        (_ui, _ub, _ud, ax, ay, w00, w01, w10,
                         w11) = umap[(i, b, d)]
                        yle, xe = geo[(i, b)]
                        ax_lo, _axh, ay_lo, _ayh = env[(i, b)]
                        img = imgs[(i, b)]

                        # ---- 4 diagonal corner-weight matrices: host
                        # precomputes them; one contiguous DMA per distinct set
                        gi = dg_unit_idx[(i, b, d)]
                        dg = dg_cache.get(gi)
                        if dg is None:
                            dg = dgpool.tile([128, 4 * 128], f32r, tag="dg")
                            nc.sync.dma_start(out=dg[:], in_=dgd[gi])
                            dg_cache[gi] = dg

                        # ---- bilinear stencil on PE (4 matmuls into PSUM)
                        psA = psa_pool.tile([128, NFREE], f32, tag="psA")
                        corners = ((0, 0), (0, 1), (1, 0), (1, 1))
                        base = (ay - ay_lo) * xe + (ax - ax_lo)
                        ifull = img[:]
                        for k, (dy_, dx_) in enumerate(corners):
                            rhs = bass.AP(
                                tensor=ifull.tensor,
                                offset=ifull.offset + base + dy_ * xe + dx_,
                                ap=[[yle * xe, 128], [xe, RY], [1, XW]])
                            nc.tensor.matmul(
                                psA[:], dg[:, k * 128:(k + 1) * 128],
                                rhs, start=(k == 0), stop=False)
                        # 5th accumulation: psA -= (ref + 1e-8)
                        nc.tensor.matmul(
                            psA[:], lhs_r(5),
                            refp_t[:, b * NFREE:(b + 1) * NFREE],
                            start=False, stop=True)

                        # ---- divergence: vol = exp(-3*|bilerp/ref - 1|)
                        r = wpool.tile([128, NFREE], f32, tag="r")
                        nc.vector.tensor_tensor(
                            out=r[:], in0=psA[:],
                            in1=refr_t[:, b * NFREE:(b + 1) * NFREE],
                            op=A.mult)
                        u32 = mybir.dt.uint32
                        nc.vector.tensor_scalar(
                            out=r[:].bitcast(u32), in0=r[:].bitcast(u32),
                            scalar1=0x7FFFFFFF, scalar2=None,
                            op0=A.bitwise_and)
                        vol = wpool.tile([128, NFREE], f32r, tag="vol")
                        nc.scalar.activation(out=vol[:], in_=r[:],
                                             func=AF.Exp, scale=-LAMBDA)

                        # ---- MLP layer 1 (feature-split halves)
                        psBC = psbc_pool.tile([128, 1024], f32, tag="psBC")
                        nc.tensor.matmul(psBC[:, 0:NFREE], lhs_r(0),
                                         vol[:], start=True, stop=True)
                        nc.tensor.matmul(psBC[:, 512:512 + NFREE], lhs_r(1),
                                         vol[:], start=True, stop=True)
                        h1 = wpool.tile([128, 2 * NFREE], f32r, tag="h1")
                        nc.vector.tensor_scalar(
                            out=h1[:, 0:NFREE], in0=psBC[:, 0:NFREE],
                            scalar1=bias_t[:, 0:1], scalar2=0.0,
                            op0=A.add, op1=A.max)
                        if (d + b + i) % 2 == 0:
                            nc.scalar.activation(
                                out=h1[:, NFREE:2 * NFREE],
                                in_=psBC[:, 512:512 + NFREE],
                                func=AF.Relu, bias=bias_t[:, 1:2], scale=1.0)
                        else:
                            nc.vector.tensor_scalar(
                                out=h1[:, NFREE:2 * NFREE],
                                in0=psBC[:, 512:512 + NFREE],
                                scalar1=bias_t[:, 1:2], scalar2=0.0,
                                op0=A.add, op1=A.max)

                        # ---- layer 2 (accumulate both feature halves)
                        psD = psd_pool.tile([128, NFREE], f32, tag="psD")
                        nc.tensor.matmul(psD[:], lhs_r(2),
                                         h1[:, 0:NFREE],
                                         start=True, stop=False)
                        nc.tensor.matmul(psD[:], lhs_r(3),
                                         h1[:, NFREE:2 * NFREE],
                                         start=False, stop=True)
                        h2 = wpool.tile([128, NFREE], f32r, tag="h2")
                        nc.scalar.activation(out=h2[:], in_=psD[:],
                                             func=AF.Relu,
                                             bias=bias_t[:, 2:3], scale=1.0)

                        # ---- layer 3 (w3 replicated 8x across partitions)
                        psE = pse_pool.tile([128, NFREE], f32, tag="psE")
                        nc.tensor.matmul(psE[:], lhs_r(4),
                                         h2[:], start=True, stop=True)
                        # sigmoid(z+b3) = 0.5*tanh((z+b3)/2) + 0.5 -- tanh
                        # lives in the exp table set (no ACT table reload)
                        th = wpool.tile([128, NFREE], f32, tag="th")
                        nc.scalar.activation(out=th[:], in_=psE[:],
                                             func=AF.Tanh,
                                             bias=bias_t[:, 3:4], scale=0.5)
                        if i == 0:
                            wrep = acc_w
                        else:
                            wrep = wpool.tile([128, NFREE], f32, tag="wrep")
                        nc.vector.tensor_scalar(
                            out=wrep[:], in0=th[:], scalar1=0.5, scalar2=0.5,
                            op0=A.mult, op1=A.add)

                        # ---- accumulate cost volume and weight sum
                        cm_eng = nc.vector if (d + i) % 3 == 0 else nc.gpsimd
                        if i == 0:
                            cm_eng.tensor_tensor(
                                out=acc_c[:], in0=vol[:].bitcast(f32),
                                in1=wrep[:], op=A.mult)
                        else:
                            tmp = wpool.tile([128, NFREE], f32, tag="tmp")
                            cm_eng.tensor_tensor(
                                out=tmp[:], in0=vol[:].bitcast(f32),
                                in1=wrep[:], op=A.mult)
                            nc.gpsimd.tensor_tensor(out=acc_c[:],
                                                    in0=acc_c[:],
                                                    in1=tmp[:], op=A.add)
                            nc.gpsimd.tensor_tensor(out=acc_w[:],
                                                    in0=acc_w[:],
                                                    in1=wrep[:], op=A.add)

                if (d % 8 == 7 or d == D - 1) and b == B - 1:
                    # ---- burst finals: out = acc_c / acc_w via exp(-ln(w)).
                    # Ln+Exp share one ACT table set; bursting every 8 depths
                    # amortizes the ~2.7us table switches.
                    for (fb, fd, f_c, f_w) in pending:
                        lnw = fpool.tile([128, NFREE], f32, tag="lnw")
                        nc.scalar.activation(out=lnw[:], in_=f_w[:],
                                             func=AF.Ln, scale=1.0)
                        rec = fpool.tile([128, NFREE], f32, tag="rec")
                        nc.scalar.activation(out=rec[:], in_=lnw[:],
                                             func=AF.Exp, scale=-1.0)
                        out_t = fpool.tile([128, NFREE], f32, tag="out_t")
                        nc.vector.tensor_tensor(out=out_t[:], in0=f_c[:],
                                                in1=rec[:], op=A.mult)
                        nc.sync.dma_start(out=outd[fb, fd], in_=out_t[:])
                    pending = []



    # this walrus build accepts only ONE sync wait per instruction: hoist
    # excess waits onto same-engine EventSemaphore ops inserted before
    if not hoist:
        return nc
    nhoist = 0
    for bb in nc.m.functions[0].blocks:
        insts = bb.instructions
        idx = 0
        while idx < len(insts):
            inst = insts[idx]
            si = inst.sync_info
            if si is not None and si.on_wait and len(si.on_wait) > 1:
                waits = list(si.on_wait)
                upds = list(si.on_update) if si.on_update else []
                inst.sync_info = mybir.SyncInfo(on_wait=waits[-1:],
                                                on_update=upds)
                for wv in waits[:-1]:
                    nop = mybir.InstEventSemaphore(
                        name=f"I-hoist-{nhoist}", ins=[], outs=[])
                    nhoist += 1
                    nop.engine = inst.engine
                    nop.sync_info = mybir.SyncInfo(on_wait=[wv], on_update=[])
                    insts.insert(idx, nop)
                    idx += 1
            idx += 1
    return nc


# ----------------------------------------------------------------------------
# host input prep + entry point
# ----------------------------------------------------------------------------

def _prep_core_inputs(features, refr_full, refp_full, env, lhs_stack,
                      bias_stack, ident):
    """Build the 8 per-core input dicts (wimg_* = halo'd per-partition
    windows sized by each view's shift envelope)."""
    from numpy.lib.stride_tricks import sliding_window_view
    pyg = max(4, max(max(abs(e[2]), abs(e[3])) for e in env.values()) + 4)
    pxg = max(8, max(max(abs(e[0]), abs(e[1])) for e in env.values()) + XW + 4)
    wp = np.zeros((NVIEW, B, C, H + 2 * pyg, W + 2 * pxg), np.float32)
    wp[:, :, :, pyg:pyg + H, pxg:pxg + W] = features[1:]
    in_maps = [dict() for _ in range(NCORES)]
    for (i, b), e in env.items():
        ax_lo, _axh, ay_lo, _ayh = e
        yle, xe = _geom(e)
        win = sliding_window_view(wp[i, b], (yle, xe), axis=(1, 2))
        yt_idx = np.arange(YT) * RY
        xh_idx = np.arange(XH) * XW
        for q in range(NCORES):
            rows = pyg + RPC * q + yt_idx + ay_lo        # [YT]
            cols = pxg + xh_idx + ax_lo                  # [XH]
            g = win[:, rows[:, None], cols[None, :]]     # [C,YT,XH,yle,xe]
            g = g.transpose(1, 2, 0, 3, 4)               # (yt,xh,c) order
            in_maps[q][f"wimg_{i}_{b}"] = np.ascontiguousarray(
                g.reshape(128, yle * xe), dtype=np.float32)
    for q in range(NCORES):
        refr_q = _tileize(refr_full[:, :, RPC * q:RPC * q + RPC, :])
        refp_q = _tileize(refp_full[:, :, RPC * q:RPC * q + RPC, :])
        in_maps[q].update({
            "refr": np.ascontiguousarray(refr_q),
            "refp": np.ascontiguousarray(refp_q),
            "mlpw": lhs_stack,
            "biasd": bias_stack,
            "identd": ident,
        })
    return in_maps


def _host_prep(inputs):
    features = inputs["features"].astype(np.float32, copy=False)
    # folded MLP weights -> block-diagonal lhsT matrices
    w1f, c1 = _fold_bn(inputs["w1"], inputs["g1"], inputs["b1"],
                       inputs["m1"], inputs["v1"])     # [16,8], [16]
    w2f, c2 = _fold_bn(inputs["w2"], inputs["g2"], inputs["b2"],
                       inputs["m2"], inputs["v2"])     # [8,16], [8]
    w3 = inputs["w3"].astype(np.float32)               # [1,8]
    b3 = inputs["b3"].astype(np.float32)               # [1]

    eye16 = np.eye(16, dtype=np.float32)
    lhs1a = np.kron(eye16, w1f[0:8, :].T)
    lhs1b = np.kron(eye16, w1f[8:16, :].T)
    lhs2a = np.kron(eye16, w2f[:, 0:8].T)
    lhs2b = np.kron(eye16, w2f[:, 8:16].T)
    lhs3 = np.kron(eye16, np.repeat(w3.T, 8, axis=1))
    negI = (-np.eye(128)).astype(np.float32)
    lhs_stack = np.ascontiguousarray(
        np.stack([lhs1a, lhs1b, lhs2a, lhs2b, lhs3, negI]))

    c1a = np.tile(c1[0:8], 16).astype(np.float32)
    c1b = np.tile(c1[8:16], 16).astype(np.float32)
    c2v = np.tile(c2, 16).astype(np.float32)
    b3h = np.full(128, 0.5 * b3[0], np.float32)      # tanh bias = b3/2
    bias_stack = np.ascontiguousarray(np.stack([c1a, c1b, c2v, b3h]))

    refp_full = (features[0] + np.float32(1e-8)).astype(np.float32)
    refr_full = (1.0 / refp_full).astype(np.float32)
    ident = np.eye(128, dtype=np.float32)
    return lhs_stack, bias_stack, refp_full, refr_full, ident


def kernel(**inputs):
    inputs = {k: np.asarray(v) for k, v in inputs.items()}
    features = inputs["features"].astype(np.float32, copy=False)
    depth_hypos = inputs["depth_hypos"].astype(np.float32, copy=False)
    ref_proj = inputs["ref_proj"].astype(np.float32, copy=False)
    src_projs = inputs["src_projs"].astype(np.float32, copy=False)

    ok, shifts = _analyze(ref_proj, src_projs, depth_hypos)
    if not ok:
        return _np_reference(**inputs)

    units = _unit_constants(shifts)
    env = _envelopes(units)
    # halo'd windows too large to keep resident -> fall back to numpy
    tot = sum(_geom(e)[0] * _geom(e)[1] * 4 for e in env.values())
    if tot > 110 * 1024:
        return _np_reference(**inputs)

    (lhs_stack, bias_stack, refp_full, refr_full,
     ident) = _host_prep(inputs)

    key = tuple((u[3], u[4], round(u[5], 9), round(u[6], 9), round(u[7], 9),
                 round(u[8], 9)) for u in units)
    prog = _COMPILE_CACHE.get(key)
    if prog is None:
        prog = _build_program(units, env)
        _COMPILE_CACHE[key] = prog

    in_maps = _prep_core_inputs(features, refr_full, refp_full, env,
                                lhs_stack, bias_stack, ident)
    # host-precomputed diagonal corner-weight matrices (f32r consumers)
    dg_plan, _dg_idx = _dg_plan(units)
    eye = np.eye(128, dtype=np.float32)
    dgd_arr = np.empty((len(dg_plan), 128, 4 * 128), np.float32)
    for n, (w00, w01, w10, w11) in enumerate(dg_plan):
        for k, wk in enumerate((w00, w01, w10, w11)):
            dgd_arr[n, :, k * 128:(k + 1) * 128] = np.float32(wk) * eye
    dgd_arr = np.ascontiguousarray(dgd_arr)
    for m in in_maps:
        m["dgd"] = dgd_arr

    import os
    from concourse.bass_utils import run_bass_kernel_spmd
    trace = bool(os.environ.get("COSTAGG_TRACE"))
    res = run_bass_kernel_spmd(prog, in_maps, core_ids=list(range(NCORES)),
                               trace=trace)
    LAST_RUN_INFO.clear()
    LAST_RUN_INFO.update({
        "exec_time_ns": res.exec_time_ns,
        "mean_exec_time_ns": res.mean_exec_time_ns,
        "profile_json": res.profile_json,
    })

    out = np.empty((B, C, D, H, W), np.float32)
    for q in range(NCORES):
        oq = res.results[q]["out"]            # [B, D, 128, NFREE]
        oq = oq.reshape(B, D, YT, XH, C, RY, XW)
        oq = oq.transpose(0, 4, 1, 2, 5, 3, 6)  # (B, C, D, yt, yl, xh, x)
        out[:, :, :, RPC * q:RPC * q + RPC, :] = oq.reshape(B, C, D, RPC, W)
    return out

